# revision 1
# baseline (speedup 1.0000x reference)
"""3-layer GAT (ogbn-arxiv shapes) on 8 Trainium2 NeuronCores.

Graph/data-parallel per the sharding hint: nodes are sharded contiguously
across the 8 cores.  Per layer each core projects its shard with an augmented
weight matrix (attention vectors ride as extra columns), AllGathers the
augmented node table, then aggregates its incoming edges: int16 dma_gather of
source rows (6 static 32768-row ranges of the table), per-edge softmax weights
on ACT/DVE, and segment-sums via one-hot matrices on the tensor engine into
per-tile PSUM accumulators.  BN+ReLU and the next layer's projection are fused
into the per-tile finalize; log_softmax at the end.

Edge->slot layout is computed on the host with a static chunk->tile-pair
assignment so the SPMD program is identical on every core.
"""

import time

import numpy as np

import concourse.bacc as bacc
import concourse.bass as bass
import concourse.mybir as mybir
import concourse.tile as tile
from concourse.bass_utils import run_bass_kernel_spmd

F32 = mybir.dt.float32
I16 = mybir.dt.int16
AF = mybir.ActivationFunctionType
OP = mybir.AluOpType

NCORES = 8

# L1 row: [h0f 128 | one | h1f 128 | one | als0 als1 | ald0 ald1 | 0...] w=320
# L2 row: [feats 256 | one | als | ald | 0...] w=320
# L3 row: [feats 40 | one | als | ald | 0...] w=64
LAYERS = [
    dict(TW=320, H=2, RW=129, ALS=258, ATT0=256, ALDB=4),
    dict(TW=320, H=1, RW=257, ALS=257, ATT0=256, ALDB=2),
    dict(TW=64, H=1, RW=41, ALS=41, ATT0=0, ALDB=42),
]


def make_cfg(n_nodes, tiles_per_core, n_ranges):
    nsh = tiles_per_core * 128
    cfg = dict(N=n_nodes, NSH=nsh, TILES=tiles_per_core,
               PAIRS=(tiles_per_core + 1) // 2, NR=n_ranges, NTOT=nsh * NCORES)
    cfg["RANGE"] = -(-cfg["NTOT"] // n_ranges)
    assert cfg["RANGE"] <= 32768
    cfg["GROUPS"] = -(-cfg["PAIRS"] // 3)
    return cfg


def group_pairs(cfg, g):
    return [p for p in range(3 * g, 3 * g + 3) if p < cfg["PAIRS"]]


def cfg_slots(cfg):
    return sum(cfg["NR"] * 256 * len(group_pairs(cfg, g)) for g in range(cfg["GROUPS"]))


# ------------------------------------------------------------------ host prep


def prepare(cfg, x, src, dst):
    N, NSH, NTOT, NR = cfg["N"], cfg["NSH"], cfg["NTOT"], cfg["NR"]
    s = src.astype(np.int64)
    d = dst.astype(np.int64)

    perm = np.arange(NTOT, dtype=np.int64)  # node -> row
    for _ in range(30):
        srow, drow = perm[s], perm[d]
        key = ((drow // NSH) * cfg["PAIRS"] + (drow % NSH) // 256) * NR + srow // cfg["RANGE"]
        cnt = np.bincount(key, minlength=NCORES * cfg["PAIRS"] * NR)
        over = np.nonzero(cnt > 256)[0]
        if len(over) == 0:
            break
        inv = np.empty(NTOT, np.int64)
        inv[perm] = np.arange(NTOT)
        cnt3 = cnt.reshape(NCORES, cfg["PAIRS"], NR)
        load = cnt3.max(axis=2)
        for k in over[:64]:
            c = k // (cfg["PAIRS"] * NR)
            p = (k // NR) % cfg["PAIRS"]
            p2 = int(np.argmin(load[c]))
            a = inv[c * NSH + p * 256]
            b = inv[c * NSH + p2 * 256]
            perm[a], perm[b] = perm[b], perm[a]
            load[c, p2] += 8
    else:
        raise RuntimeError("pair balancing failed")

    srow, drow = perm[s], perm[d]
    core = drow // NSH
    pair = (drow % NSH) // 256
    rng = srow // cfg["RANGE"]

    G = cfg["GROUPS"]
    blk_off = np.zeros(cfg["PAIRS"], np.int64)
    call_off = np.zeros((G, NR), np.int64)
    off = 0
    for g in range(G):
        bp = group_pairs(cfg, g)
        for j, p in enumerate(bp):
            blk_off[p] = 256 * j
        for r in range(NR):
            call_off[g, r] = off
            off += 256 * len(bp)
    slots = off
    assert slots == cfg_slots(cfg)

    gidx = np.zeros((NCORES, slots), np.int16)
    didx = np.zeros((NCORES, slots), np.int16)
    dstloc = np.full((NCORES, slots), -1.0, np.float32)

    order = np.lexsort((drow, rng, pair, core))
    so, do_, co, po, ro = srow[order], drow[order], core[order], pair[order], rng[order]
    keyo = (co * cfg["PAIRS"] + po) * NR + ro
    chg = np.empty(len(keyo), bool)
    if len(keyo) == 0:
        chg = chg
    else:
        chg[0] = True
    chg[1:] = keyo[1:] != keyo[:-1]
    if len(keyo):
        sidx = np.nonzero(chg)[0]
        pos = np.arange(len(keyo)) - np.repeat(sidx, np.diff(np.append(sidx, len(keyo))))
        assert pos.max() < 256
        sl = call_off[po // 3, ro] + blk_off[po] + pos
        gidx[co, sl] = (so - ro * cfg["RANGE"]).astype(np.int16)
        didx[co, sl] = (do_ - co * NSH).astype(np.int16)
        dstloc[co, sl] = (do_ - (co * NSH + po * 256)).astype(np.float32)

    def wrap(a):  # index i -> [i%16, i//16], replicated to 128 partitions
        ncol = a.shape[1] // 16
        w = a.reshape(a.shape[0], ncol, 16).transpose(0, 2, 1)
        return np.ascontiguousarray(np.tile(w, (1, 8, 1)))

    nch = slots // 128
    dstloc_c = np.ascontiguousarray(dstloc.reshape(NCORES, nch, 128).transpose(0, 2, 1))

    inv = np.empty(NTOT, np.int64)
    inv[perm] = np.arange(NTOT)
    xsh = np.zeros((NCORES, NSH, x.shape[1]), np.float32)
    for c in range(NCORES):
        rows = inv[c * NSH : (c + 1) * NSH]
        real = rows < N
        xsh[c][real] = x[rows[real]]
    xT = np.ascontiguousarray(xsh.transpose(0, 2, 1))
    return wrap(gidx), wrap(didx), dstloc_c, xT, perm


def prep_weights(inp):
    EPS = 1e-5
    HID = inp["as1"].shape[1]
    w1 = np.zeros((inp["W1"].shape[0], 320), np.float32)
    w1[:, 0:128] = inp["W1"][:, :HID]
    w1[:, 129:257] = inp["W1"][:, HID:]
    w1[:, 258] = inp["W1"][:, :HID] @ inp["as1"][0]
    w1[:, 259] = inp["W1"][:, HID:] @ inp["as1"][1]
    w1[:, 260] = inp["W1"][:, :HID] @ inp["ad1"][0]
    w1[:, 261] = inp["W1"][:, HID:] @ inp["ad1"][1]
    w2 = np.zeros((256, 320), np.float32)
    w2[:, 0:256] = inp["W2"]
    w2[:, 257] = inp["W2"] @ inp["as2"][0]
    w2[:, 258] = inp["W2"] @ inp["ad2"][0]
    w3 = np.zeros((256, 64), np.float32)
    w3[:, 0:40] = inp["W3"]
    w3[:, 41] = inp["W3"] @ inp["as3"][0]
    w3[:, 42] = inp["W3"] @ inp["ad3"][0]

    def fold(b, g, be, m, v):
        k = g / np.sqrt(v + EPS)
        return k.astype(np.float32), ((b - m) * k + be).astype(np.float32)

    A1, B1 = fold(inp["b1"], inp["g1"], inp["be1"], inp["m1"], inp["v1"])
    A2, B2 = fold(inp["b2"], inp["g2"], inp["be2"], inp["m2"], inp["v2"])
    rep = lambda a: np.ascontiguousarray(np.tile(a[None, :], (128, 1)))
    return dict(w1=w1, w2=w2, w3=w3, A1=rep(A1), B1=rep(B1), A2=rep(A2),
                B2=rep(B2), b3=rep(inp["b3"].astype(np.float32)),
                iota=rep(np.arange(256, dtype=np.float32)),
                pidx=np.ascontiguousarray(
                    np.arange(128, dtype=np.float32)[:, None]),
                ident=np.eye(128, dtype=np.float32))


# ------------------------------------------------------------------ builder


def build(cfg, passes=1):
    NSH, TILES, NR, G = cfg["NSH"], cfg["TILES"], cfg["NR"], cfg["GROUPS"]
    SLOTS = cfg_slots(cfg)
    nc = bacc.Bacc()
    ext = lambda n, sh, dt=F32: nc.dram_tensor(n, sh, dt, kind="ExternalInput")
    D = dict(
        xT=ext("xT", [128, NSH]), w1=ext("w1", [128, 320]),
        w2=ext("w2", [256, 320]), w3=ext("w3", [256, 64]),
        A1=ext("A1", [128, 256]), B1=ext("B1", [128, 256]),
        A2=ext("A2", [128, 256]), B2=ext("B2", [128, 256]),
        b3=ext("b3", [128, 40]), iota=ext("iota", [128, 256]),
        pidx=ext("pidx", [128, 1]),
        ident=ext("ident", [128, 128]),
        gidx=ext("gidx", [128, SLOTS // 16], I16),
        didx=ext("didx", [128, SLOTS // 16], I16),
        dstloc=ext("dstloc", [128, SLOTS // 128]),
    )
    out = nc.dram_tensor("out", [NSH, 40], F32, kind="ExternalOutput")
    dbgG = nc.dram_tensor("dbgG", [128, 6 * 320], F32, kind="ExternalOutput")
    haug = [nc.dram_tensor(f"haug{l}", [NSH, LAYERS[l]["TW"]], F32) for l in range(3)]
    tabs = [nc.dram_tensor(f"tab{l}", [cfg["NTOT"], LAYERS[l]["TW"]], F32,
                           addr_space="Shared") for l in range(3)]

    with tile.TileContext(nc) as tc:
        with (
            tc.tile_pool(name="res", bufs=1) as res,
            tc.tile_pool(name="gp", bufs=2) as gp,
            tc.tile_pool(name="wp", bufs=3) as wp,
            tc.tile_pool(name="pt", bufs=1, space="PSUM") as pt,
            tc.tile_pool(name="pz", bufs=1, space="PSUM") as pz,
            tc.tile_pool(name="pagg", bufs=1, space="PSUM") as pagg,
        ):
            R = {}
            for n, sh, dt in (
                ("w1", [128, 320], F32), ("w2", [128, 640], F32),
                ("w3", [128, 128], F32), ("A1", [128, 256], F32),
                ("B1", [128, 256], F32), ("A2", [128, 256], F32),
                ("B2", [128, 256], F32), ("b3", [128, 40], F32),
                ("iota", [128, 256], F32), ("ident", [128, 128], F32),
                ("pidx", [128, 1], F32),
                ("gidx", [128, SLOTS // 16], I16),
                ("didx", [128, SLOTS // 16], I16),
                ("dstloc", [128, SLOTS // 128], F32),
            ):
                R[n] = res.tile(sh, dt, name=n, tag=n)
                if n in ("w2", "w3"):
                    w = sh[1] // 2
                    for k in range(2):
                        nc.scalar.dma_start(
                            out=R[n][:, k * w : (k + 1) * w],
                            in_=D[n][k * 128 : (k + 1) * 128, :])
                else:
                    nc.scalar.dma_start(out=R[n][:], in_=D[n][:])

            # ---- layer-1 projection
            for _rep in range(passes):
              for t in range(TILES):
                xt = wp.tile([128, 128], F32, tag="xt")
                nc.scalar.dma_start(out=xt[:], in_=D["xT"][:, t * 128 : (t + 1) * 128])
                ps = pz.tile([128, 320], F32, tag="proj")
                nc.tensor.matmul(ps[:], lhsT=xt[:], rhs=R["w1"][:], start=True, stop=True)
                hs = wp.tile([128, 320], F32, tag="hs")
                nc.scalar.activation(hs[:], ps[:], AF.Copy)
                nc.vector.memset(hs[:, 128:129], 1.0)
                nc.vector.memset(hs[:, 257:258], 1.0)
                nc.scalar.dma_start(out=haug[0][t * 128 : (t + 1) * 128, :], in_=hs[:])
              for l in range(3):
                nc.gpsimd.collective_compute(
                    "AllGather", OP.bypass,
                    ins=[haug[l][:].opt()], outs=[tabs[l][:].opt()],
                    replica_groups=[list(range(NCORES))])
                edge_phase(nc, cfg, l, R, out, haug, tabs, gp, wp, pt, pz, pagg, dbgG)
    nc.compile()
    return nc


def edge_phase(nc, cfg, l, R, out, haug, tabs, gp, wp, pt, pz, pagg, dbgG=None):
    L = LAYERS[l]
    TW, H, RW = L["TW"], L["H"], L["RW"]
    NR, G, TILES = cfg["NR"], cfg["GROUPS"], cfg["TILES"]
    PW = H * RW  # psum width used
    call16 = 0
    ccol0 = 0
    for g in range(G):
        pairs = group_pairs(cfg, g)
        ns = 256 * len(pairs)
        nb = ns // 128
        # gathers
        Gt = []
        for r in range(NR):
            gt = gp.tile([128, 6 * TW], F32, tag=f"G{r}")
            lo = r * cfg["RANGE"]
            hi = min(lo + cfg["RANGE"], cfg["NTOT"])
            nc.gpsimd.dma_gather(
                out_ap=gt[:, : nb * TW].rearrange("p (b t) -> p b t", b=nb),
                in_ap=tabs[l][lo:hi, :],
                idxs_ap=R["gidx"][:, call16 + r * (ns // 16) : call16 + (r + 1) * (ns // 16)],
                num_idxs=ns, num_idxs_reg=ns, elem_size=TW, single_packet=False)
            if l == 0 and g == 0 and r == 0 and dbgG is not None:
                nc.scalar.dma_start(out=dbgG[:, : nb * TW], in_=gt[:, : nb * TW])
            Gt.append(gt)
        ad = gp.tile([128, 6 * NR * 64], F32, tag="ald")
        nc.gpsimd.dma_gather(
            out_ap=ad[:, : NR * nb * 64].rearrange("p (b t) -> p b t", b=NR * nb),
            in_ap=haug[l][:, L["ATT0"] : L["ATT0"] + 64],
            idxs_ap=R["didx"][:, call16 : call16 + NR * (ns // 16)],
            num_idxs=NR * ns, num_idxs_reg=NR * ns, elem_size=64,
            elem_step=TW, single_packet=False)
        # per-edge weights exp(lrelu(als[src] + ald[dst]))
        exw = []
        for r in range(NR):
            ex = wp.tile([128, 12], F32, tag=f"ex{r}")
            gv = Gt[r][:, : nb * TW].rearrange("p (b t) -> p b t", b=nb)
            av = ad[:, : NR * nb * 64].rearrange("p (b t) -> p b t", b=NR * nb)
            ev = ex[:, : nb * H].rearrange("p (b t) -> p b t", b=nb)
            nc.vector.tensor_tensor(
                out=ev, in0=gv[:, :, L["ALS"] : L["ALS"] + H],
                in1=av[:, r * nb : (r + 1) * nb, L["ALDB"] : L["ALDB"] + H],
                op=OP.add)
            ex2 = wp.tile([128, 12], F32, tag=f"ex2_{r}")
            nc.vector.tensor_scalar(out=ex2[:, : nb * H], in0=ex[:, : nb * H],
                                    scalar1=0.2, scalar2=None, op0=OP.mult)
            nc.vector.tensor_tensor(out=ex[:, : nb * H], in0=ex[:, : nb * H],
                                    in1=ex2[:, : nb * H], op=OP.max)
            nc.scalar.activation(ex[:, : nb * H], ex[:, : nb * H], AF.Exp)
            if l == 0 and g == 0 and r == 0 and dbgG is not None:
                nc.scalar.dma_start(out=dbgG[:, 1600 : 1600 + nb * H], in_=ex[:, : nb * H])
            exw.append(ex)
        # chunk matmuls into per-tile psums
        ptile = {}
        for jp, p in enumerate(pairs):
            for side in range(2):
                t = 2 * p + side
                if t < TILES:
                    ptile[t] = pagg.tile([128, 272], F32, name=f"agg_t{t}", tag=f"agg{t % 6}")
        started = set()
        for jp, p in enumerate(pairs):
            for r in range(NR):
                for side in range(2):
                    b = 2 * jp + side
                    ccol = ccol0 + r * nb + b
                    for h in range(H):
                        s2 = wp.tile([128, 256], F32, tag="s2")
                        nc.vector.tensor_scalar(
                            out=s2[:], in0=R["iota"][:],
                            scalar1=R["dstloc"][:, ccol : ccol + 1],
                            scalar2=exw[r][:, b * H + h : b * H + h + 1],
                            op0=OP.is_equal, op1=OP.mult)
                        if l == 0:
                            rhs = Gt[r][:, b * TW + h * 129 : b * TW + h * 129 + RW]
                        else:
                            rhs = Gt[r][:, b * TW : b * TW + RW]
                        for ti in range(2):
                            t = 2 * p + ti
                            if t >= TILES:
                                continue
                            nc.tensor.matmul(
                                ptile[t][:, h * RW : (h + 1) * RW],
                                lhsT=s2[:, ti * 128 : (ti + 1) * 128], rhs=rhs,
                                start=t not in started, stop=False,
                                skip_group_check=True)
                            started.add(t)
        # self-loop chunk per tile (tile's own rows, diagonal S), then finalize
        for jp, p in enumerate(pairs):
            for side in range(2):
                t = 2 * p + side
                if t >= TILES:
                    continue
                ht = wp.tile([128, TW], F32, tag="ht")
                nc.scalar.dma_start(out=ht[:, 0:TW],
                                    in_=haug[l][t * 128 : (t + 1) * 128, :])
                exs = wp.tile([128, 2], F32, tag="exs")
                nc.vector.tensor_tensor(
                    out=exs[:, 0:H], in0=ht[:, L["ALS"] : L["ALS"] + H],
                    in1=ht[:, L["ALS"] + H : L["ALS"] + 2 * H], op=OP.add)
                exs2 = wp.tile([128, 2], F32, tag="exs2")
                nc.vector.tensor_scalar(out=exs2[:, 0:H], in0=exs[:, 0:H],
                                        scalar1=0.2, scalar2=None, op0=OP.mult)
                nc.vector.tensor_tensor(out=exs[:, 0:H], in0=exs[:, 0:H],
                                        in1=exs2[:, 0:H], op=OP.max)
                nc.scalar.activation(exs[:, 0:H], exs[:, 0:H], AF.Exp)
                for h in range(H):
                    ss = wp.tile([128, 128], F32, tag="ss")
                    nc.vector.tensor_scalar(
                        out=ss[:], in0=R["iota"][:, 0:128],
                        scalar1=R["pidx"][:, 0:1],
                        scalar2=exs[:, h : h + 1],
                        op0=OP.is_equal, op1=OP.mult)
                    if l == 0:
                        rhs = ht[:, h * 129 : h * 129 + RW]
                    else:
                        rhs = ht[:, 0:RW]
                    nc.tensor.matmul(
                        ptile[t][:, h * RW : (h + 1) * RW], lhsT=ss[:], rhs=rhs,
                        start=t not in started, stop=h == H - 1,
                        skip_group_check=True)
                    started.add(t)
                finalize_tile(nc, cfg, l, t, ptile[t], R, out, haug, wp, pt, pz)
        call16 += NR * ns // 16
        ccol0 += NR * nb


def finalize_tile(nc, cfg, l, t, ps, R, out, haug, wp, pt, pz):
    L = LAYERS[l]
    H, RW = L["H"], L["RW"]
    rows = slice(t * 128, (t + 1) * 128)
    if l < 2:
        rc = wp.tile([128, 2], F32, tag="rc")
        if l == 0:  # den at cols 128 and 257
            nc.vector.reciprocal(rc[:, 0:1], ps[:, 128:129])
            nc.vector.reciprocal(rc[:, 1:2], ps[:, 257:258])
        else:
            nc.vector.reciprocal(rc[:, 0:1], ps[:, 256:257])
        z = wp.tile([128, 256], F32, tag="z")
        if l == 0:
            nc.vector.tensor_scalar(out=z[:, 0:128], in0=ps[:, 0:128],
                                    scalar1=rc[:, 0:1], scalar2=None, op0=OP.mult)
            nc.vector.tensor_scalar(out=z[:, 128:256], in0=ps[:, 129:257],
                                    scalar1=rc[:, 1:2], scalar2=None, op0=OP.mult)
        else:
            nc.vector.tensor_scalar(out=z[:], in0=ps[:, 0:256],
                                    scalar1=rc[:, 0:1], scalar2=None, op0=OP.mult)
        A, B = ("A1", "B1") if l == 0 else ("A2", "B2")
        nc.vector.tensor_tensor(out=z[:], in0=z[:], in1=R[A][:], op=OP.mult)
        nc.vector.tensor_tensor(out=z[:], in0=z[:], in1=R[B][:], op=OP.add)
        nc.scalar.activation(z[:], z[:], AF.Relu)
        # fused next-layer projection: haug[l+1][t] = z @ w_next
        zt_ps = pt.tile([128, 256], F32, tag="zt")
        for k in range(2):
            nc.tensor.transpose(zt_ps[:, k * 128 : (k + 1) * 128],
                                z[:, k * 128 : (k + 1) * 128], R["ident"][:])
        zt = wp.tile([128, 256], F32, tag="zts")
        nc.scalar.activation(zt[:], zt_ps[:], AF.Copy)
        wn, w = ("w2", 320) if l == 0 else ("w3", 64)
        pp = pz.tile([128, 320], F32, tag="proj")
        for k in range(2):
            nc.tensor.matmul(pp[:, 0:w], lhsT=zt[:, k * 128 : (k + 1) * 128],
                             rhs=R[wn][:, k * w : (k + 1) * w],
                             start=k == 0, stop=k == 1)
        hs = wp.tile([128, 320], F32, tag="hs")
        nc.scalar.activation(hs[:, 0:w], pp[:, 0:w], AF.Copy)
        if l == 0:
            nc.vector.memset(hs[:, 256:257], 1.0)
        else:
            nc.vector.memset(hs[:, 40:41], 1.0)
        nc.scalar.dma_start(out=haug[l + 1][rows, :], in_=hs[:, 0:w])
    else:
        # out = feats/den + b3, then log_softmax
        rc = wp.tile([128, 2], F32, tag="rc")
        nc.vector.reciprocal(rc[:, 0:1], ps[:, 40:41])
        o = wp.tile([128, 40], F32, tag="o")
        nc.vector.tensor_scalar(out=o[:], in0=ps[:, 0:40], scalar1=rc[:, 0:1],
                                scalar2=None, op0=OP.mult)
        nc.vector.tensor_tensor(out=o[:], in0=o[:], in1=R["b3"][:], op=OP.add)
        nmx = wp.tile([128, 1], F32, tag="nmx")
        nc.vector.tensor_reduce(out=nmx[:], in_=o[:], op=OP.max,
                                axis=mybir.AxisListType.X, negate=True)
        tmp = wp.tile([128, 40], F32, tag="tmp")
        se = wp.tile([128, 1], F32, tag="se")
        nc.scalar.activation(tmp[:], o[:], AF.Exp, bias=nmx[:, 0:1], accum_out=se[:])
        lse = wp.tile([128, 1], F32, tag="lse")
        nc.scalar.activation(lse[:], se[:], AF.Ln)
        o2 = wp.tile([128, 40], F32, tag="o2")
        nc.vector.tensor_scalar(out=o2[:], in0=o[:], scalar1=nmx[:, 0:1],
                                scalar2=lse[:, 0:1], op0=OP.add, op1=OP.subtract)
        nc.scalar.dma_start(out=out[t * 128 : (t + 1) * 128, :], in_=o2[:])


# ------------------------------------------------------------------ entry


_CACHE = {}
LAST_TIMES = []


def kernel(**inputs):
    return kernel_cfg(make_cfg(169343, 166, 6), **inputs)


def kernel_cfg(cfg, passes=1, **inputs):
    x = np.asarray(inputs["x"], np.float32)
    src = np.asarray(inputs["src"])
    dst = np.asarray(inputs["dst"])
    gidx, didx, dstloc, xT, perm = prepare(cfg, x, src, dst)
    W = prep_weights({k: np.asarray(v) for k, v in inputs.items()})
    key = (cfg["NSH"], passes)
    if key not in _CACHE:
        _CACHE[key] = build(cfg, passes)
    nc = _CACHE[key]
    in_maps = []
    for c in range(NCORES):
        m = dict(W)
        m["xT"] = xT[c]
        m["gidx"] = gidx[c]
        m["didx"] = didx[c]
        m["dstloc"] = dstloc[c]
        in_maps.append(m)
    t0 = time.time()
    res = run_bass_kernel_spmd(nc, in_maps, core_ids=list(range(NCORES)))
    LAST_TIMES.append(time.time() - t0)
    big = np.concatenate([res.results[c]["out"] for c in range(NCORES)], 0)
    return big[perm[: cfg["N"]]].astype(np.float32)



# revision 2
# speedup vs baseline: 1.0966x; 1.0966x over previous
"""3-layer GAT (ogbn-arxiv shapes) on 8 Trainium2 NeuronCores.

Nodes sharded contiguously across cores (21504/core, 168 tiles of 128).
Per layer: project shard into an augmented bf16 table (attention logit
columns ride as extra cols), AllGather the table, then per 128-dst tile
aggregate incoming edges: slots bucketed per (tile, src-range) with cap
128, gathered in one dma_gather per (tile-group, range); per-slot softmax
weights on DVE/ACT; weighted one-hot S matrices (DVE/Pool) drive one
[128x128]x[128xRHS] seg-sum matmul per chunk into a per-tile PSUM
accumulator.  Self-loop + 1/den + folded-BN bias are fused into the
finalize (BN scale folded into next layer's W host-side), followed by PE
transpose and the next layer's projection.  log_softmax at the end.
"""

import time

import ml_dtypes
import numpy as np

import concourse.bacc as bacc
import concourse.mybir as mybir
import concourse.tile as tile
from concourse.bass_utils import run_bass_kernel_spmd

F32 = mybir.dt.float32
BF16 = mybir.dt.bfloat16
I16 = mybir.dt.int16
AF = mybir.ActivationFunctionType
OP = mybir.AluOpType

NCORES = 8
NSH = 21504          # nodes per core (168 tiles)
T = 168
GT = 8               # tiles per group
NGR = 21             # groups
NR = 6               # src ranges (int16 gather window)
RANGE = 28672
NTOT = NSH * NCORES  # 172032
CAP = 128            # slots per (tile, range)
SLOTS = T * NR * CAP           # 129024 per core
NCH = SLOTS // 128             # 1008 chunks
N = 169343
EPS = 1e-5

# layer col layouts (bf16 table width TW; RHS = seg-matmul rhs width)
LAY = [
    dict(TW=384, H=2, RHS=258, ALS=258, ALD=260, C0=256, APOS=4, DEN=128),
    dict(TW=384, H=1, RHS=257, ALS=257, ALD=258, C0=256, APOS=2, DEN=256),
    dict(TW=128, H=1, RHS=41, ALS=41, ALD=42, C0=0, APOS=42, DEN=40),
]


def _wrap(a):  # [NC, S] -> [NC, 128, S//16] ; idx i -> [i%16, i//16], tiled x8
    nc_, s = a.shape
    w = a.reshape(nc_, s // 16, 16).transpose(0, 2, 1)
    return np.ascontiguousarray(np.tile(w, (1, 8, 1)))


def prepare(x, src, dst):
    s = np.asarray(src, np.int64)
    d = np.asarray(dst, np.int64)
    core = d // NSH
    tl = (d % NSH) // 128
    g = tl // GT
    t = tl % GT
    r = s // RANGE
    bucket = (core * T + tl) * NR + r
    order = np.argsort(bucket, kind="stable")
    bo = bucket[order]
    chg = np.ones(len(bo), bool)
    chg[1:] = bo[1:] != bo[:-1]
    sidx = np.nonzero(chg)[0]
    pos = np.arange(len(bo)) - np.repeat(sidx, np.diff(np.append(sidx, len(bo))))
    if pos.max() >= CAP:
        raise RuntimeError(f"bucket overflow: {pos.max()}")
    so, do_, co = s[order], d[order], core[order]
    go, to, ro = g[order], t[order], r[order]
    s_id = ((go * NR + ro) * GT + to) * 128 + pos

    gidx = np.zeros((NCORES, SLOTS), np.int16)
    didx = np.zeros((NCORES, SLOTS), np.int16)
    dloc = np.full((NCORES, SLOTS), -1.0, np.float32)
    gidx[co, s_id] = (so - ro * RANGE).astype(np.int16)
    didx[co, s_id] = (do_ % NSH).astype(np.int16)
    dloc[co, s_id] = (do_ % 128).astype(np.float32)

    dloc_c = np.ascontiguousarray(
        dloc.reshape(NCORES, NCH, 128).transpose(0, 2, 1))

    xsh = np.zeros((NCORES, NSH, x.shape[1]), np.float32)
    flat = np.asarray(x, np.float32)
    for c in range(NCORES):
        lo, hi = c * NSH, min((c + 1) * NSH, N)
        if hi > lo:
            xsh[c, : hi - lo] = flat[lo:hi]
    xT = np.ascontiguousarray(xsh.transpose(0, 2, 1)).astype(ml_dtypes.bfloat16)
    return _wrap(gidx), _wrap(didx), dloc_c, xT


def prep_weights(inp):
    bf = ml_dtypes.bfloat16
    W1, W2, W3 = inp["W1"], inp["W2"], inp["W3"]

    def fold(b, gm, be, m, v):
        A = gm / np.sqrt(v + EPS)
        B = (b - m) * A + be
        return A.astype(np.float32), (B / A).astype(np.float32)

    A1, BA1 = fold(inp["b1"], inp["g1"], inp["be1"], inp["m1"], inp["v1"])
    A2, BA2 = fold(inp["b2"], inp["g2"], inp["be2"], inp["m2"], inp["v2"])
    W2p = (W2 * A1[:, None]).astype(np.float32)
    W3p = (W3 * A2[:, None]).astype(np.float32)

    w1 = np.zeros((128, 384), np.float32)
    w1[:, 0:128] = W1[:, 0:128]
    w1[:, 129:257] = W1[:, 128:256]
    w1[:, 258] = W1[:, 0:128] @ inp["as1"][0]
    w1[:, 259] = W1[:, 128:256] @ inp["as1"][1]
    w1[:, 260] = W1[:, 0:128] @ inp["ad1"][0]
    w1[:, 261] = W1[:, 128:256] @ inp["ad1"][1]
    w2 = np.zeros((256, 384), np.float32)
    w2[:, 0:256] = W2p
    w2[:, 257] = W2p @ inp["as2"][0]
    w2[:, 258] = W2p @ inp["ad2"][0]
    w3 = np.zeros((256, 128), np.float32)
    w3[:, 0:40] = W3p
    w3[:, 41] = W3p @ inp["as3"][0]
    w3[:, 42] = W3p @ inp["ad3"][0]

    rep = lambda a: np.ascontiguousarray(np.tile(a[None, :], (128, 1))).astype(np.float32)
    return dict(
        w1=w1.astype(bf), w2=w2.astype(bf), w3=w3.astype(bf),
        BA1=rep(BA1), BA2=rep(BA2), b3r=rep(inp["b3"].astype(np.float32)),
        iota=rep(np.arange(128, dtype=np.float32)),
        ident=np.eye(128, dtype=np.float32),
    )


def build(passes=1):
    nc = bacc.Bacc()
    ext = lambda n, sh, dt: nc.dram_tensor(n, sh, dt, kind="ExternalInput")
    D = dict(
        xT=ext("xT", [128, NSH], BF16),
        w1=ext("w1", [128, 384], BF16), w2=ext("w2", [256, 384], BF16),
        w3=ext("w3", [256, 128], BF16),
        BA1=ext("BA1", [128, 256], F32), BA2=ext("BA2", [128, 256], F32),
        b3r=ext("b3r", [128, 40], F32), iota=ext("iota", [128, 128], F32),
        ident=ext("ident", [128, 128], F32),
        gidx=ext("gidx", [128, SLOTS // 16], I16),
        didx=ext("didx", [128, SLOTS // 16], I16),
        dloc=ext("dloc", [128, NCH], F32),
    )
    out = nc.dram_tensor("out", [NSH, 40], F32, kind="ExternalOutput")
    TWs = [LAY[0]["TW"], LAY[1]["TW"], LAY[2]["TW"]]
    haug = [nc.dram_tensor(f"haug{l}", [NSH, TWs[l]], BF16) for l in range(3)]
    tabs = [nc.dram_tensor(f"tab{l}", [NTOT, TWs[l]], BF16, addr_space="Shared")
            for l in range(3)]

    with tile.TileContext(nc) as tc:
        with (
            tc.tile_pool(name="res", bufs=1) as res,
            tc.tile_pool(name="gb", bufs=2) as gb,
            tc.tile_pool(name="wb", bufs=2) as wb,
            tc.tile_pool(name="sp", bufs=4) as sp,
            tc.tile_pool(name="wp", bufs=3) as wp,
            tc.tile_pool(name="pagg", bufs=2, space="PSUM") as pagg,
            tc.tile_pool(name="pzt", bufs=2, space="PSUM") as pzt,
            tc.tile_pool(name="ppj", bufs=2, space="PSUM") as ppj,
        ):
            R = {}
            for nm, sh, dt in (
                ("w1", [128, 384], BF16), ("BA1", [128, 256], F32),
                ("BA2", [128, 256], F32), ("b3r", [128, 40], F32),
                ("iota", [128, 128], F32), ("ident", [128, 128], F32),
                ("gidx", [128, SLOTS // 16], I16),
                ("didx", [128, SLOTS // 16], I16),
                ("dloc", [128, NCH], F32),
            ):
                R[nm] = res.tile(sh, dt, name=nm, tag=nm)
                nc.sync.dma_start(out=R[nm][:], in_=D[nm][:])
            for nm, w in (("w2", 384), ("w3", 128)):
                R[nm] = res.tile([128, 2 * w], BF16, name=nm, tag=nm)
                for k in range(2):
                    nc.sync.dma_start(out=R[nm][:, k * w : (k + 1) * w],
                                      in_=D[nm][k * 128 : (k + 1) * 128, :])

            for _rep in range(passes):
                # ---- layer-1 projection into haug[0]
                for t in range(T):
                    xt = wp.tile([128, 128], BF16, tag="xt")
                    nc.sync.dma_start(out=xt[:], in_=D["xT"][:, t * 128 : (t + 1) * 128])
                    pp = ppj.tile([128, 384], F32, tag="pj")
                    nc.tensor.matmul(pp[:], lhsT=xt[:], rhs=R["w1"][:],
                                     start=True, stop=True)
                    hs = wp.tile([128, 384], BF16, tag="hs")
                    nc.scalar.activation(hs[:], pp[:], AF.Copy)
                    nc.vector.memset(hs[:, 128:129], 1.0)
                    nc.vector.memset(hs[:, 257:258], 1.0)
                    nc.sync.dma_start(out=haug[0][t * 128 : (t + 1) * 128, :], in_=hs[:])
                for l in range(3):
                    nc.gpsimd.collective_compute(
                        "AllGather", OP.bypass,
                        ins=[haug[l][:].opt()], outs=[tabs[l][:].opt()],
                        replica_groups=[list(range(NCORES))])
                    edge_phase(nc, l, R, out, haug, tabs, gb, wb, sp, wp,
                               pagg, pzt, ppj)
    nc.compile()
    return nc


def edge_phase(nc, l, R, out, haug, tabs, gb, wb, sp, wp, pagg, pzt, ppj):
    L = LAY[l]
    TW, H, RHS = L["TW"], L["H"], L["RHS"]
    NB = NR * GT  # 48 blocks per group
    for g in range(NGR):
        G = gb.tile([128, NB * TW], BF16, tag="G")
        for r in range(NR):
            lo = r * RANGE
            nc.gpsimd.dma_gather(
                out_ap=G[:, r * GT * TW : (r + 1) * GT * TW].rearrange(
                    "p (b t) -> p b t", b=GT),
                in_ap=tabs[l][lo : lo + RANGE, :],
                idxs_ap=R["gidx"][:, (g * NR + r) * 64 : (g * NR + r + 1) * 64],
                num_idxs=GT * 128, num_idxs_reg=GT * 128, elem_size=TW,
                single_packet=False)
        ad = gb.tile([128, NB * 128], BF16, tag="ad")
        nc.gpsimd.dma_gather(
            out_ap=ad[:].rearrange("p (b t) -> p b t", b=NB),
            in_ap=haug[l][:, L["C0"] : L["C0"] + 128],
            idxs_ap=R["didx"][:, g * 384 : (g + 1) * 384],
            num_idxs=NB * 128, num_idxs_reg=NB * 128, elem_size=128,
            elem_step=TW, single_packet=False)

        # per-slot weights w = exp(leakyrelu(als_src + ald_dst)) ; [128, NB, H]
        ew = wb.tile([128, NB * H], F32, tag="ew")
        ewv = ew[:].rearrange("p (b h) -> p b h", b=NB)
        gv = G[:].rearrange("p (b t) -> p b t", b=NB)
        av = ad[:].rearrange("p (b t) -> p b t", b=NB)
        nc.vector.tensor_tensor(out=ewv, in0=gv[:, :, L["ALS"] : L["ALS"] + H],
                                in1=av[:, :, L["APOS"] : L["APOS"] + H], op=OP.add)
        nc.vector.scalar_tensor_tensor(out=ew[:], in0=ew[:], scalar=0.2,
                                       in1=ew[:], op0=OP.mult, op1=OP.max)
        nc.scalar.activation(ew[:], ew[:], AF.Exp)
        if l == 0:
            rr = wb.tile([128, NB], F32, tag="rr")
            nc.vector.reciprocal(rr[:], ewv[:, :, 0:1])
            nc.vector.tensor_tensor(out=rr[:], in0=rr[:], in1=ewv[:, :, 1:2],
                                    op=OP.mult)
            for b in range(NB):
                eng = nc.vector if b % 2 else nc.gpsimd
                eng.tensor_scalar(
                    out=G[:, b * TW + 129 : b * TW + 258],
                    in0=G[:, b * TW + 129 : b * TW + 258],
                    scalar1=rr[:, b : b + 1], scalar2=None, op0=OP.mult)

        # self rows + self weights for this group's 8 tiles
        ht = wb.tile([128, GT * TW], BF16, tag="ht")
        rows = haug[l][g * GT * 128 : (g + 1) * GT * 128, :]
        nc.sync.dma_start(out=ht[:].rearrange("p (b t) -> p b t", b=GT),
                          in_=rows.rearrange("(b p) t -> p b t", p=128))
        htv = ht[:].rearrange("p (b t) -> p b t", b=GT)
        ews = wb.tile([128, GT * H], F32, tag="ews")
        ewsv = ews[:].rearrange("p (b h) -> p b h", b=GT)
        nc.vector.tensor_tensor(out=ewsv, in0=htv[:, :, L["ALS"] : L["ALS"] + H],
                                in1=htv[:, :, L["ALD"] : L["ALD"] + H], op=OP.add)
        nc.vector.scalar_tensor_tensor(out=ews[:], in0=ews[:], scalar=0.2,
                                       in1=ews[:], op0=OP.mult, op1=OP.max)
        nc.scalar.activation(ews[:], ews[:], AF.Exp)

        for t in range(GT):
            ps = pagg.tile([128, 320], F32, tag="agg")
            for r in range(NR):
                b = r * GT + t
                ch = (g * NR + r) * GT + t
                S = sp.tile([128, 128], BF16, tag=f"S{r % 4}")
                eng = nc.vector if r % 2 else nc.gpsimd
                eng.tensor_scalar(
                    out=S[:], in0=R["iota"][:],
                    scalar1=R["dloc"][:, ch : ch + 1],
                    scalar2=ew[:, b * H : b * H + 1],
                    op0=OP.is_equal, op1=OP.mult)
                nc.tensor.matmul(ps[:, 0:RHS], lhsT=S[:],
                                 rhs=G[:, b * TW : b * TW + RHS],
                                 start=r == 0, stop=r == NR - 1,
                                 skip_group_check=True)
            finalize(nc, l, g, t, ps, htv, ews, R, out, haug, wp, pzt, ppj)


def finalize(nc, l, g, t, ps, htv, ews, R, out, haug, wp, pzt, ppj):
    L = LAY[l]
    RHS = L["RHS"]
    tg = g * GT + t
    rows = slice(tg * 128, (tg + 1) * 128)
    t1 = wp.tile([128, RHS], F32, tag="t1")
    if l == 0:
        nc.vector.scalar_tensor_tensor(
            out=t1[:, 0:129], in0=htv[:, t, 0:129],
            scalar=ews[:, 2 * t : 2 * t + 1], in1=ps[:, 0:129],
            op0=OP.mult, op1=OP.add)
        nc.vector.scalar_tensor_tensor(
            out=t1[:, 129:258], in0=htv[:, t, 129:258],
            scalar=ews[:, 2 * t + 1 : 2 * t + 2], in1=ps[:, 129:258],
            op0=OP.mult, op1=OP.add)
    else:
        nc.vector.scalar_tensor_tensor(
            out=t1[:], in0=htv[:, t, 0:RHS],
            scalar=ews[:, t : t + 1], in1=ps[:, 0:RHS],
            op0=OP.mult, op1=OP.add)
    rc = wp.tile([128, 2], F32, tag="rc")
    nden = 2 if l == 0 else 1
    t1v = t1[:].rearrange("p (a b) -> p a b", a=nden)
    nc.vector.reciprocal(rc[:, 0:nden], t1v[:, :, L["DEN"] : L["DEN"] + 1])

    if l < 2:
        z = wp.tile([128, 256], F32, tag="z")
        BA = R["BA1"] if l == 0 else R["BA2"]
        if l == 0:
            nc.vector.scalar_tensor_tensor(
                out=z[:, 0:128], in0=t1[:, 0:128], scalar=rc[:, 0:1],
                in1=BA[:, 0:128], op0=OP.mult, op1=OP.add)
            nc.vector.scalar_tensor_tensor(
                out=z[:, 128:256], in0=t1[:, 129:257], scalar=rc[:, 1:2],
                in1=BA[:, 128:256], op0=OP.mult, op1=OP.add)
        else:
            nc.vector.scalar_tensor_tensor(
                out=z[:], in0=t1[:, 0:256], scalar=rc[:, 0:1],
                in1=BA[:], op0=OP.mult, op1=OP.add)
        zt = wp.tile([128, 256], BF16, tag="zt")
        for k in range(2):
            zp = pzt.tile([128, 128], F32, tag=f"zt{k}")
            nc.tensor.transpose(zp[:], z[:, k * 128 : (k + 1) * 128], R["ident"][:])
            nc.scalar.activation(zt[:, k * 128 : (k + 1) * 128], zp[:], AF.Relu)
        wn, TWn = ("w2", 384) if l == 0 else ("w3", 128)
        pp = ppj.tile([128, 384], F32, tag="pj")
        for k in range(2):
            nc.tensor.matmul(pp[:, 0:TWn], lhsT=zt[:, k * 128 : (k + 1) * 128],
                             rhs=R[wn][:, k * TWn : (k + 1) * TWn],
                             start=k == 0, stop=k == 1)
        hs = wp.tile([128, 384], BF16, tag="hs")
        nc.scalar.activation(hs[:, 0:TWn], pp[:, 0:TWn], AF.Copy)
        onec = 256 if l == 0 else 40
        nc.vector.memset(hs[:, onec : onec + 1], 1.0)
        nc.sync.dma_start(out=haug[l + 1][rows, :], in_=hs[:, 0:TWn])
    else:
        o = wp.tile([128, 40], F32, tag="o")
        nc.vector.scalar_tensor_tensor(
            out=o[:], in0=t1[:, 0:40], scalar=rc[:, 0:1], in1=R["b3r"][:],
            op0=OP.mult, op1=OP.add)
        nmx = wp.tile([128, 1], F32, tag="nmx")
        nc.vector.tensor_reduce(out=nmx[:], in_=o[:], op=OP.max,
                                axis=mybir.AxisListType.X, negate=True)
        tmp = wp.tile([128, 40], F32, tag="tmp")
        se = wp.tile([128, 1], F32, tag="se")
        nc.scalar.activation(tmp[:], o[:], AF.Exp, bias=nmx[:, 0:1], accum_out=se[:])
        lse = wp.tile([128, 1], F32, tag="lse")
        nc.scalar.activation(lse[:], se[:], AF.Ln)
        o2 = wp.tile([128, 40], F32, tag="o2")
        nc.vector.tensor_scalar(out=o2[:], in0=o[:], scalar1=nmx[:, 0:1],
                                scalar2=lse[:, 0:1], op0=OP.add, op1=OP.subtract)
        nc.sync.dma_start(out=out[rows, :], in_=o2[:])


_CACHE = {}
LAST_TIMES = []


def kernel(**inputs):
    return kernel_cfg(passes=1, **inputs)


def kernel_cfg(passes=1, **inputs):
    x = np.asarray(inputs["x"], np.float32)
    gidx, didx, dloc, xT = prepare(x, inputs["src"], inputs["dst"])
    W = prep_weights({k: np.asarray(v) for k, v in inputs.items()})
    if passes not in _CACHE:
        _CACHE[passes] = build(passes)
    nc = _CACHE[passes]
    in_maps = []
    for c in range(NCORES):
        m = dict(W)
        m["xT"] = xT[c]
        m["gidx"] = gidx[c]
        m["didx"] = didx[c]
        m["dloc"] = dloc[c]
        in_maps.append(m)
    t0 = time.time()
    res = run_bass_kernel_spmd(nc, in_maps, core_ids=list(range(NCORES)))
    LAST_TIMES.append(time.time() - t0)
    big = np.concatenate([res.results[c]["out"] for c in range(NCORES)], 0)
    return big[:N].astype(np.float32)
